# revision 17
# baseline (speedup 1.0000x reference)
"""MoE (top-2 of 8 experts + shared expert, SwiGLU) on 8 trn2 NeuronCores.

Sharding: pure data-parallel over tokens. Each core takes 512 of the 4096
tokens and computes the router, all 8 routed experts (dense, exactly the
reference formulation: non-selected experts get a 0.0 gate), the shared
expert and the final sigmoid mix for its shard. Weights are replicated,
pre-cast to bf16 on the host. No collectives.

Layout: activations live transposed ([feature, token]) on chip, so every
matmul consumes weights straight from DRAM as the stationary [K, M] operand
and no transpose instruction is ever needed. The host pre-transposes the x
shard and post-transposes the output shard.

The per-expert combine sum_e gate_e * expert_e is folded into PSUM: the gate
is broadcast along partitions (K=1 matmul against a ones row), multiplied
into the SwiGLU activation, and all experts' down-projections accumulate
into 6 pinned PSUM banks. The remaining 2 banks ping-pong the gate/up
matmul outputs.
"""

import numpy as np
from contextlib import ExitStack

import concourse.bass as bass
import concourse.mybir as mybir
import concourse.tile as tile
from concourse import bacc
from concourse.bass_utils import run_bass_kernel_spmd
from concourse.masks import make_identity

B, S, D = 4, 1024, 768
E, H, HS = 8, 768, 3072
N_CORES = 8
T = (B * S) // N_CORES  # 512 tokens per core
P = 128
KD = D // P    # 6 k-tiles over d_model
MH = H // P    # 6 m-tiles over expert hidden
MS = HS // P   # 24 m-tiles over shared hidden
TM = T // P    # 4 token tiles (router layout)
F32 = mybir.dt.float32
BF16 = mybir.dt.bfloat16
NEG_BIG = -1e30

Alu = mybir.AluOpType
Act = mybir.ActivationFunctionType
AX = mybir.AxisListType


def _build_program(repeat=1, skeleton=False):
    nc = bacc.Bacc("TRN2", target_bir_lowering=False, debug=False,
                   num_devices=N_CORES)

    xt = nc.dram_tensor("xt", [D, T], F32, kind="ExternalInput")
    xbi = nc.dram_tensor("xbi", [D, T], BF16, kind="ExternalInput")
    rw = nc.dram_tensor("rw", [D, E], F32, kind="ExternalInput")
    sgw = nc.dram_tensor("sgw", [D, 1], F32, kind="ExternalInput")
    sgb = nc.dram_tensor("sgb", [1, 1], F32, kind="ExternalInput")
    wg = nc.dram_tensor("wg", [E, D, H], BF16, kind="ExternalInput")
    wu = nc.dram_tensor("wu", [E, D, H], BF16, kind="ExternalInput")
    wd = nc.dram_tensor("wd", [E, H, D], BF16, kind="ExternalInput")
    wsg = nc.dram_tensor("wsg", [D, HS], BF16, kind="ExternalInput")
    wsu = nc.dram_tensor("wsu", [D, HS], BF16, kind="ExternalInput")
    wsd = nc.dram_tensor("wsd", [HS, D], BF16, kind="ExternalInput")
    out_t = nc.dram_tensor("out_t", [D, T], F32, kind="ExternalOutput")

    with tile.TileContext(nc) as tc, ExitStack() as ctx:
        if repeat > 1:
            ctx.enter_context(tc.For_i(0, repeat, 1))
        const = ctx.enter_context(tc.tile_pool(name="const", bufs=1))
        ident = const.tile([P, P], F32, tag="ident")
        make_identity(nc, ident)
        ones_b = const.tile([1, P], BF16, tag="ones_b")
        nc.vector.memset(ones_b[:], 1.0)
        ones_f = const.tile([1, P], F32, tag="ones_f")
        nc.vector.memset(ones_f[:], 1.0)

        # ---- small weights ----
        smallp = ctx.enter_context(tc.tile_pool(name="small", bufs=1))
        rws = []
        for k in range(KD):
            t_ = smallp.tile([P, E], F32, tag=f"rw{k}")
            nc.sync.dma_start(t_[:], rw[k * P:(k + 1) * P, :])
            rws.append(t_)
        sgws = []
        for k in range(KD):
            t_ = smallp.tile([P, 1], F32, tag=f"sgw{k}")
            nc.sync.dma_start(t_[:], sgw[k * P:(k + 1) * P, :])
            sgws.append(t_)
        sgbt = smallp.tile([1, 1], F32, tag="sgb")
        nc.sync.dma_start(sgbt[:], sgb[:, :])

        # ---- long-lived activations ----
        gbcp = ctx.enter_context(tc.tile_pool(name="gbc", bufs=E))
        abcp = ctx.enter_context(tc.tile_pool(name="abc", bufs=1))
        shp = ctx.enter_context(tc.tile_pool(name="shared", bufs=KD))
        outp = ctx.enter_context(tc.tile_pool(name="outsb", bufs=2))
        onep = ctx.enter_context(tc.tile_pool(name="oneoff", bufs=1))
        xbpool = ctx.enter_context(tc.tile_pool(name="xb", bufs=KD))

        # =====================================================
        # x load + bf16 cast; router + alpha (need f32 x, scoped)
        # =====================================================
        gT_bf = onep.tile([E, T], BF16, tag="gTb")
        a_bc = abcp.tile([P, T], F32, tag="abc")
        xbs = []
        with tc.tile_pool(name="x", bufs=KD) as xpool, \
             tc.tile_pool(name="psum_l", bufs=2, space="PSUM") as plp, \
             tc.tile_pool(name="psum_gT", bufs=2, space="PSUM") as pgtp, \
             tc.tile_pool(name="rsb", bufs=2) as rsb:
            xts = []
            for k in range(KD):
                t_ = xpool.tile([P, T], F32, tag="xt")
                nc.sync.dma_start(t_[:], xt[k * P:(k + 1) * P, :])
                b_ = xbpool.tile([P, T], BF16, tag="xb")
                nc.sync.dma_start(b_[:], xbi[k * P:(k + 1) * P, :])
                xts.append(t_)
                xbs.append(b_)
            for m in range(TM):
                pl = plp.tile([P, E], F32, tag="pl")
                for k in range(KD):
                    nc.tensor.matmul(
                        pl[:], xts[k][:, m * P:(m + 1) * P], rws[k][:],
                        start=(k == 0), stop=(k == KD - 1))
                # top-2 renormalized softmax gates, dense over E
                m1 = rsb.tile([P, 1], F32, tag="m1")
                nc.vector.reduce_max(m1[:], pl[:], AX.X)
                nm1 = rsb.tile([P, 1], F32, tag="nm1")
                nc.vector.tensor_scalar_mul(nm1[:], m1[:], -1.0)
                eexp = rsb.tile([P, E], F32, tag="eexp")
                nc.scalar.activation(eexp[:], pl[:], Act.Exp, bias=nm1[:])
                eq = rsb.tile([P, E], F32, tag="eq")
                nc.vector.tensor_scalar(eq[:], pl[:], m1[:], None, Alu.is_equal)
                masked = rsb.tile([P, E], F32, tag="masked")
                nc.vector.scalar_tensor_tensor(
                    masked[:], eq[:], NEG_BIG, pl[:], Alu.mult, Alu.add)
                m2 = rsb.tile([P, 1], F32, tag="m2")
                nc.vector.reduce_max(m2[:], masked[:], AX.X)
                ge = rsb.tile([P, E], F32, tag="ge")
                nc.vector.tensor_scalar(ge[:], pl[:], m2[:], None, Alu.is_ge)
                gsel = rsb.tile([P, E], F32, tag="gsel")
                nc.vector.tensor_tensor(gsel[:], eexp[:], ge[:], Alu.mult)
                den = rsb.tile([P, 1], F32, tag="den")
                nc.vector.reduce_sum(den[:], gsel[:], AX.X)
                rden = rsb.tile([P, 1], F32, tag="rden")
                nc.vector.reciprocal(rden[:], den[:])
                dg = rsb.tile([P, E], F32, tag="dg")
                nc.vector.tensor_scalar(dg[:], gsel[:], rden[:], None, Alu.mult)
                # transpose [128 tok, E] -> [E, 128 tok] slice of [E, T]
                pt = pgtp.tile([E, P], F32, tag="gTm")
                nc.tensor.transpose(pt[:], dg[:], ident[:])
                nc.vector.tensor_copy(gT_bf[:, m * P:(m + 1) * P], pt[:])

            # alpha = sigmoid(x @ sg_w + sg_b), broadcast to [P, T]
            pa = plp.tile([1, T], F32, tag="pa")
            for k in range(KD):
                nc.tensor.matmul(pa[:], sgws[k][:], xts[k][:],
                                 start=(k == 0), stop=(k == KD - 1))
            arow = onep.tile([1, T], F32, tag="arow")
            nc.scalar.activation(arow[:], pa[:], Act.Sigmoid, bias=sgbt[:])
            pab = pgtp.tile([P, T], F32, tag="pab")
            nc.tensor.matmul(pab[:], ones_f[:], arow[:], start=True, stop=True)
            nc.vector.tensor_copy(a_bc[:], pab[:])

        # flatten expert rows to partition 0 so they are legal matmul rhs
        g_flat = onep.tile([1, E * T], BF16, tag="gflat")
        for e in range(E):
            nc.sync.dma_start(g_flat[0:1, e * T:(e + 1) * T],
                              gT_bf[e:e + 1, :])

        # per-expert gate rows broadcast to [P, T] (K=1 matmul w/ ones)
        g_bcs = []
        with tc.tile_pool(name="psum_bc", bufs=2, space="PSUM") as pbcp:
            for e in range(E):
                pb = pbcp.tile([P, T], F32, tag="pb")
                nc.tensor.matmul(pb[:], ones_b[:],
                                 g_flat[0:1, e * T:(e + 1) * T],
                                 start=True, stop=True)
                gb = gbcp.tile([P, T], BF16, tag="gbc")
                nc.vector.tensor_copy(gb[:], pb[:])
                g_bcs.append(gb)

        # =====================================================
        # Shared expert SwiGLU activation As = silu(x@wsg) * (x@wsu)
        # (two HS/2 halves to bound SBUF), then down-proj into acc6
        # =====================================================
        HS2 = HS // 2
        MS2 = MS // 2
        as_pool = ctx.enter_context(tc.tile_pool(name="as", bufs=MS))
        as_tiles = []
        with tc.tile_pool(name="wsh", bufs=KD) as wshp, \
             tc.tile_pool(name="psum_gs", bufs=1, space="PSUM") as pgs, \
             tc.tile_pool(name="psum_us", bufs=1, space="PSUM") as pus, \
             tc.tile_pool(name="sgu", bufs=2) as sgup:
            for half in range(2):
                wsg_t, wsu_t = [], []
                for k in range(KD):
                    t_ = wshp.tile([P, HS2], BF16, tag="wsg")
                    nc.sync.dma_start(
                        t_[:], wsg[k * P:(k + 1) * P,
                                   half * HS2:(half + 1) * HS2])
                    wsg_t.append(t_)
                    t_ = wshp.tile([P, HS2], BF16, tag="wsu")
                    nc.sync.dma_start(
                        t_[:], wsu[k * P:(k + 1) * P,
                                   half * HS2:(half + 1) * HS2])
                    wsu_t.append(t_)
                for j in range(MS2):
                    pg = pgs.tile([P, T], F32, tag="pg")
                    for k in range(KD):
                        nc.tensor.matmul(
                            pg[:], wsg_t[k][:, j * P:(j + 1) * P], xbs[k][:],
                            start=(k == 0), stop=(k == KD - 1))
                    pu = pus.tile([P, T], F32, tag="pu")
                    for k in range(KD):
                        nc.tensor.matmul(
                            pu[:], wsu_t[k][:, j * P:(j + 1) * P], xbs[k][:],
                            start=(k == 0), stop=(k == KD - 1))
                    if skeleton:
                        a_ = as_pool.tile([P, T], BF16, tag="as")
                        nc.vector.tensor_copy(a_[0:1, 0:16], pg[0:1, 0:16])
                        nc.vector.tensor_copy(a_[0:1, 16:32], pu[0:1, 0:16])
                        as_tiles.append(xbs[j % KD])
                        continue
                    sg = sgup.tile([P, T], BF16, tag="sg")
                    nc.scalar.activation(sg[:], pg[:], Act.Sigmoid)
                    us = sgup.tile([P, T], BF16, tag="us")
                    nc.vector.tensor_copy(us[:], pu[:])
                    gu = sgup.tile([P, T], BF16, tag="gu")
                    nc.vector.tensor_tensor(gu[:], pg[:], us[:], Alu.mult)
                    a_ = as_pool.tile([P, T], BF16, tag="as")
                    nc.vector.tensor_tensor(a_[:], sg[:], gu[:], Alu.mult)
                    as_tiles.append(a_)

        # shared down-proj: k-outer accumulation into 6 pinned banks,
        # evicted to SBUF f32.
        shared_sb = []
        with tc.tile_pool(name="acc6", bufs=KD, space="PSUM") as acc_pool, \
             tc.tile_pool(name="wsd", bufs=10) as wsdp:
            acc = [acc_pool.tile([P, T], F32, tag="acc", name=f"acc{d}")
                   for d in range(KD)]
            for k in range(MS):
                t_ = wsdp.tile([P, D], BF16, tag="wsd")
                nc.sync.dma_start(t_[:], wsd[k * P:(k + 1) * P, :])
                for d in range(KD):
                    nc.tensor.matmul(
                        acc[d][:], t_[:, d * P:(d + 1) * P], as_tiles[k][:],
                        start=(k == 0), stop=(k == MS - 1))
            for d in range(KD):
                s_ = shp.tile([P, T], F32, tag="sh")
                nc.vector.tensor_copy(s_[:], acc[d][:])
                shared_sb.append(s_)

        # =====================================================
        # Routed experts: per expert SwiGLU (gate folded in); per
        # (expert, d) down-proj groups churn through PSUM and are
        # accumulated into SBUF f32 tiles by the DVE.
        # =====================================================
        racp = ctx.enter_context(tc.tile_pool(name="racc", bufs=KD))
        racc = [racp.tile([P, T], F32, tag="racc", name=f"racc{d}")
                for d in range(KD)]
        with tc.tile_pool(name="wge", bufs=10) as wgep, \
             tc.tile_pool(name="wue", bufs=10) as wuep, \
             tc.tile_pool(name="wde", bufs=10) as wdep, \
             tc.tile_pool(name="psum_g", bufs=2, space="PSUM") as pgp, \
             tc.tile_pool(name="psum_u", bufs=2, space="PSUM") as pup, \
             tc.tile_pool(name="psum_d", bufs=3, space="PSUM") as pdp, \
             tc.tile_pool(name="gu", bufs=2) as gup, \
             tc.tile_pool(name="a2", bufs=MH + 2) as a2p:
            for e in range(E):
                wg_t, wu_t, wd_t = [], [], []
                for k in range(KD):
                    t_ = wgep.tile([P, H], BF16, tag="wge")
                    nc.sync.dma_start(t_[:], wg[e, k * P:(k + 1) * P, :])
                    wg_t.append(t_)
                    t_ = wuep.tile([P, H], BF16, tag="wue")
                    nc.sync.dma_start(t_[:], wu[e, k * P:(k + 1) * P, :])
                    wu_t.append(t_)
                    t_ = wdep.tile([P, D], BF16, tag="wde")
                    nc.sync.dma_start(t_[:], wd[e, k * P:(k + 1) * P, :])
                    wd_t.append(t_)
                a2_tiles = []
                for h in range(MH):
                    pg = pgp.tile([P, T], F32, tag="pg")
                    for k in range(KD):
                        nc.tensor.matmul(
                            pg[:], wg_t[k][:, h * P:(h + 1) * P], xbs[k][:],
                            start=(k == 0), stop=(k == KD - 1))
                    pu = pup.tile([P, T], F32, tag="pu")
                    for k in range(KD):
                        nc.tensor.matmul(
                            pu[:], wu_t[k][:, h * P:(h + 1) * P], xbs[k][:],
                            start=(k == 0), stop=(k == KD - 1))
                    if skeleton:
                        a2 = a2p.tile([P, T], BF16, tag="a2")
                        nc.vector.tensor_copy(a2[0:1, 0:16], pg[0:1, 0:16])
                        nc.vector.tensor_copy(a2[0:1, 16:32], pu[0:1, 0:16])
                        a2_tiles.append(xbs[h % KD])
                        continue
                    sg = gup.tile([P, T], BF16, tag="sg")
                    nc.scalar.activation(sg[:], pg[:], Act.Sigmoid)
                    us = gup.tile([P, T], BF16, tag="us")
                    nc.vector.tensor_tensor(us[:], pu[:], g_bcs[e][:],
                                            Alu.mult)
                    gu = gup.tile([P, T], BF16, tag="gu")
                    nc.vector.tensor_tensor(gu[:], pg[:], us[:], Alu.mult)
                    a2 = a2p.tile([P, T], BF16, tag="a2")
                    nc.vector.tensor_tensor(a2[:], sg[:], gu[:], Alu.mult)
                    a2_tiles.append(a2)
                for d in range(KD):
                    pd = pdp.tile([P, T], F32, tag="pd")
                    for k in range(MH):
                        nc.tensor.matmul(
                            pd[:], wd_t[k][:, d * P:(d + 1) * P],
                            a2_tiles[k][:],
                            start=(k == 0), stop=(k == MH - 1))
                    if e == 0:
                        nc.vector.tensor_copy(racc[d][:], pd[:])
                    else:
                        nc.vector.tensor_tensor(racc[d][:], pd[:],
                                                racc[d][:], Alu.add)

        # =====================================================
        # out = routed + alpha * (shared - routed)
        # =====================================================
        finp = ctx.enter_context(tc.tile_pool(name="fin", bufs=2))
        for d in range(KD):
            t1 = finp.tile([P, T], F32, tag="t1")
            nc.vector.tensor_tensor(t1[:], shared_sb[d][:], racc[d][:],
                                    Alu.subtract)
            t2 = finp.tile([P, T], F32, tag="t2")
            nc.vector.tensor_tensor(t2[:], t1[:], a_bc[:], Alu.mult)
            o_ = outp.tile([P, T], F32, tag="o")
            nc.vector.tensor_tensor(o_[:], t2[:], racc[d][:], Alu.add)
            nc.sync.dma_start(out_t[d * P:(d + 1) * P, :], o_[:])

    nc.compile()
    return nc


_NC_CACHE = None


def _get_program():
    global _NC_CACHE
    if _NC_CACHE is None:
        _NC_CACHE = _build_program()
    return _NC_CACHE


def make_in_maps(x, router_w, w_gate, w_up, w_down, ws_gate, ws_up, ws_down,
                 sg_w, sg_b):
    bf = mybir.dt.np(BF16)
    f32 = np.float32
    x2 = np.asarray(x, dtype=f32).reshape(B * S, D)
    shared = {
        "rw": np.asarray(router_w, dtype=f32),
        "sgw": np.asarray(sg_w, dtype=f32).reshape(D, 1),
        "sgb": np.asarray(sg_b, dtype=f32).reshape(1, 1),
        "wg": np.asarray(w_gate, dtype=f32).astype(bf),
        "wu": np.asarray(w_up, dtype=f32).astype(bf),
        "wd": np.asarray(w_down, dtype=f32).astype(bf),
        "wsg": np.asarray(ws_gate, dtype=f32).astype(bf),
        "wsu": np.asarray(ws_up, dtype=f32).astype(bf),
        "wsd": np.asarray(ws_down, dtype=f32).astype(bf),
    }
    in_maps = []
    for c in range(N_CORES):
        m = dict(shared)
        xtc = np.ascontiguousarray(x2[c * T:(c + 1) * T, :].T)
        m["xt"] = xtc
        m["xbi"] = xtc.astype(bf)
        in_maps.append(m)
    return in_maps


def assemble_out(results):
    cols = [np.asarray(results[c]["out_t"]) for c in range(N_CORES)]
    full_t = np.concatenate(cols, axis=1)  # [D, B*S]
    return np.ascontiguousarray(full_t.T).reshape(B, S, D).astype(np.float32)


def kernel(**inputs) -> np.ndarray:
    nc = _get_program()
    in_maps = make_in_maps(**inputs)
    res = run_bass_kernel_spmd(nc, in_maps, list(range(N_CORES)))
    return assemble_out(res.results)


# revision 19
# speedup vs baseline: 1.0062x; 1.0062x over previous
"""MoE (top-2 of 8 experts + shared expert, SwiGLU) on 8 trn2 NeuronCores.

Sharding: pure data-parallel over tokens. Each core takes 512 of the 4096
tokens and computes the router, all 8 routed experts (dense, exactly the
reference formulation: non-selected experts get a 0.0 gate), the shared
expert and the final sigmoid mix for its shard. Weights are replicated,
pre-cast to bf16 on the host. No collectives.

Layout: activations live transposed ([feature, token]) on chip, so every
matmul consumes weights straight from DRAM as the stationary [K, M] operand
and no transpose instruction is ever needed. The host pre-transposes the x
shard and post-transposes the output shard.

The per-expert combine sum_e gate_e * expert_e is folded into PSUM: the gate
is broadcast along partitions (K=1 matmul against a ones row), multiplied
into the SwiGLU activation, and all experts' down-projections accumulate
into 6 pinned PSUM banks. The remaining 2 banks ping-pong the gate/up
matmul outputs.
"""

import numpy as np
from contextlib import ExitStack

import concourse.bass as bass
import concourse.mybir as mybir
import concourse.tile as tile
from concourse import bacc
from concourse.bass_utils import run_bass_kernel_spmd
from concourse.masks import make_identity

B, S, D = 4, 1024, 768
E, H, HS = 8, 768, 3072
N_CORES = 8
T = (B * S) // N_CORES  # 512 tokens per core
P = 128
KD = D // P    # 6 k-tiles over d_model
MH = H // P    # 6 m-tiles over expert hidden
MS = HS // P   # 24 m-tiles over shared hidden
TM = T // P    # 4 token tiles (router layout)
F32 = mybir.dt.float32
BF16 = mybir.dt.bfloat16
NEG_BIG = -1e30

Alu = mybir.AluOpType
Act = mybir.ActivationFunctionType
AX = mybir.AxisListType


def _build_program(repeat=1, skeleton=False, no_wdma=False):
    nc = bacc.Bacc("TRN2", target_bir_lowering=False, debug=False,
                   num_devices=N_CORES)

    xt = nc.dram_tensor("xt", [D, T], F32, kind="ExternalInput")
    xbi = nc.dram_tensor("xbi", [D, T], BF16, kind="ExternalInput")
    rw = nc.dram_tensor("rw", [D, E], F32, kind="ExternalInput")
    sgw = nc.dram_tensor("sgw", [D, 1], F32, kind="ExternalInput")
    sgb = nc.dram_tensor("sgb", [1, 1], F32, kind="ExternalInput")
    wg = nc.dram_tensor("wg", [E, D, H], BF16, kind="ExternalInput")
    wu = nc.dram_tensor("wu", [E, D, H], BF16, kind="ExternalInput")
    wd = nc.dram_tensor("wd", [E, H, D], BF16, kind="ExternalInput")
    wsg = nc.dram_tensor("wsg", [D, HS], BF16, kind="ExternalInput")
    wsu = nc.dram_tensor("wsu", [D, HS], BF16, kind="ExternalInput")
    wsd = nc.dram_tensor("wsd", [HS, D], BF16, kind="ExternalInput")
    out_t = nc.dram_tensor("out_t", [D, T], F32, kind="ExternalOutput")

    with tile.TileContext(nc) as tc, ExitStack() as ctx:
        if repeat > 1:
            ctx.enter_context(tc.For_i(0, repeat, 1))
        const = ctx.enter_context(tc.tile_pool(name="const", bufs=1))
        ident = const.tile([P, P], F32, tag="ident")
        make_identity(nc, ident)
        ones_b = const.tile([1, P], BF16, tag="ones_b")
        nc.vector.memset(ones_b[:], 1.0)
        ones_f = const.tile([1, P], F32, tag="ones_f")
        nc.vector.memset(ones_f[:], 1.0)

        # ---- small weights ----
        smallp = ctx.enter_context(tc.tile_pool(name="small", bufs=1))
        rws = []
        for k in range(KD):
            t_ = smallp.tile([P, E], F32, tag=f"rw{k}")
            nc.sync.dma_start(t_[:], rw[k * P:(k + 1) * P, :])
            rws.append(t_)
        sgws = []
        for k in range(KD):
            t_ = smallp.tile([P, 1], F32, tag=f"sgw{k}")
            nc.sync.dma_start(t_[:], sgw[k * P:(k + 1) * P, :])
            sgws.append(t_)
        sgbt = smallp.tile([1, 1], F32, tag="sgb")
        nc.sync.dma_start(sgbt[:], sgb[:, :])

        # ---- long-lived activations ----
        gbcp = ctx.enter_context(tc.tile_pool(name="gbc", bufs=E))
        abcp = ctx.enter_context(tc.tile_pool(name="abc", bufs=1))
        shp = ctx.enter_context(tc.tile_pool(name="shared", bufs=KD))
        outp = ctx.enter_context(tc.tile_pool(name="outsb", bufs=2))
        onep = ctx.enter_context(tc.tile_pool(name="oneoff", bufs=1))
        xbpool = ctx.enter_context(tc.tile_pool(name="xb", bufs=KD))

        # =====================================================
        # x load + bf16 cast; router + alpha (need f32 x, scoped)
        # =====================================================
        gT_bf = onep.tile([E, T], BF16, tag="gTb")
        a_bc = abcp.tile([P, T], F32, tag="abc")
        xbs = []
        with tc.tile_pool(name="x", bufs=KD) as xpool, \
             tc.tile_pool(name="psum_l", bufs=2, space="PSUM") as plp, \
             tc.tile_pool(name="psum_gT", bufs=2, space="PSUM") as pgtp, \
             tc.tile_pool(name="rsb", bufs=2) as rsb:
            xts = []
            for k in range(KD):
                t_ = xpool.tile([P, T], F32, tag="xt")
                nc.sync.dma_start(t_[:], xt[k * P:(k + 1) * P, :])
                b_ = xbpool.tile([P, T], BF16, tag="xb")
                nc.sync.dma_start(b_[:], xbi[k * P:(k + 1) * P, :])
                xts.append(t_)
                xbs.append(b_)
            for m in range(TM):
                pl = plp.tile([P, E], F32, tag="pl")
                for k in range(KD):
                    nc.tensor.matmul(
                        pl[:], xts[k][:, m * P:(m + 1) * P], rws[k][:],
                        start=(k == 0), stop=(k == KD - 1))
                # top-2 renormalized softmax gates, dense over E
                m1 = rsb.tile([P, 1], F32, tag="m1")
                nc.vector.reduce_max(m1[:], pl[:], AX.X)
                nm1 = rsb.tile([P, 1], F32, tag="nm1")
                nc.vector.tensor_scalar_mul(nm1[:], m1[:], -1.0)
                eexp = rsb.tile([P, E], F32, tag="eexp")
                nc.scalar.activation(eexp[:], pl[:], Act.Exp, bias=nm1[:])
                eq = rsb.tile([P, E], F32, tag="eq")
                nc.vector.tensor_scalar(eq[:], pl[:], m1[:], None, Alu.is_equal)
                masked = rsb.tile([P, E], F32, tag="masked")
                nc.vector.scalar_tensor_tensor(
                    masked[:], eq[:], NEG_BIG, pl[:], Alu.mult, Alu.add)
                m2 = rsb.tile([P, 1], F32, tag="m2")
                nc.vector.reduce_max(m2[:], masked[:], AX.X)
                ge = rsb.tile([P, E], F32, tag="ge")
                nc.vector.tensor_scalar(ge[:], pl[:], m2[:], None, Alu.is_ge)
                gsel = rsb.tile([P, E], F32, tag="gsel")
                nc.vector.tensor_tensor(gsel[:], eexp[:], ge[:], Alu.mult)
                den = rsb.tile([P, 1], F32, tag="den")
                nc.vector.reduce_sum(den[:], gsel[:], AX.X)
                rden = rsb.tile([P, 1], F32, tag="rden")
                nc.vector.reciprocal(rden[:], den[:])
                dg = rsb.tile([P, E], F32, tag="dg")
                nc.vector.tensor_scalar(dg[:], gsel[:], rden[:], None, Alu.mult)
                # transpose [128 tok, E] -> [E, 128 tok] slice of [E, T]
                pt = pgtp.tile([E, P], F32, tag="gTm")
                nc.tensor.transpose(pt[:], dg[:], ident[:])
                nc.vector.tensor_copy(gT_bf[:, m * P:(m + 1) * P], pt[:])

            # alpha = sigmoid(x @ sg_w + sg_b), broadcast to [P, T]
            pa = plp.tile([1, T], F32, tag="pa")
            for k in range(KD):
                nc.tensor.matmul(pa[:], sgws[k][:], xts[k][:],
                                 start=(k == 0), stop=(k == KD - 1))
            arow = onep.tile([1, T], F32, tag="arow")
            nc.scalar.activation(arow[:], pa[:], Act.Sigmoid, bias=sgbt[:])
            pab = pgtp.tile([P, T], F32, tag="pab")
            nc.tensor.matmul(pab[:], ones_f[:], arow[:], start=True, stop=True)
            nc.vector.tensor_copy(a_bc[:], pab[:])

        # flatten expert rows to partition 0 so they are legal matmul rhs
        g_flat = onep.tile([1, E * T], BF16, tag="gflat")
        for e in range(E):
            nc.sync.dma_start(g_flat[0:1, e * T:(e + 1) * T],
                              gT_bf[e:e + 1, :])

        # per-expert gate rows broadcast to [P, T] (K=1 matmul w/ ones)
        g_bcs = []
        with tc.tile_pool(name="psum_bc", bufs=2, space="PSUM") as pbcp:
            for e in range(E):
                pb = pbcp.tile([P, T], F32, tag="pb")
                nc.tensor.matmul(pb[:], ones_b[:],
                                 g_flat[0:1, e * T:(e + 1) * T],
                                 start=True, stop=True)
                gb = gbcp.tile([P, T], BF16, tag="gbc")
                nc.vector.tensor_copy(gb[:], pb[:])
                g_bcs.append(gb)

        # =====================================================
        # Shared expert SwiGLU activation As = silu(x@wsg) * (x@wsu)
        # (two HS/2 halves to bound SBUF), then down-proj into acc6
        # =====================================================
        HS2 = HS // 2
        MS2 = MS // 2
        as_pool = ctx.enter_context(tc.tile_pool(name="as", bufs=MS))
        as_tiles = []
        with tc.tile_pool(name="wsh", bufs=KD) as wshp, \
             tc.tile_pool(name="psum_gs", bufs=1, space="PSUM") as pgs, \
             tc.tile_pool(name="psum_us", bufs=1, space="PSUM") as pus, \
             tc.tile_pool(name="sgu", bufs=2) as sgup:
            wsg_t0, wsu_t0 = None, None
            for half in range(2):
                if no_wdma and half == 1:
                    wsg_t, wsu_t = wsg_t0, wsu_t0
                else:
                    wsg_t, wsu_t = [], []
                    for k in range(KD):
                        t_ = wshp.tile([P, HS2], BF16, tag="wsg")
                        nc.sync.dma_start(
                            t_[:], wsg[k * P:(k + 1) * P,
                                       half * HS2:(half + 1) * HS2])
                        wsg_t.append(t_)
                        t_ = wshp.tile([P, HS2], BF16, tag="wsu")
                        nc.sync.dma_start(
                            t_[:], wsu[k * P:(k + 1) * P,
                                       half * HS2:(half + 1) * HS2])
                        wsu_t.append(t_)
                    wsg_t0, wsu_t0 = wsg_t, wsu_t
                for j in range(MS2):
                    pg = pgs.tile([P, T], F32, tag="pg")
                    for k in range(KD):
                        nc.tensor.matmul(
                            pg[:], wsg_t[k][:, j * P:(j + 1) * P], xbs[k][:],
                            start=(k == 0), stop=(k == KD - 1))
                    pu = pus.tile([P, T], F32, tag="pu")
                    for k in range(KD):
                        nc.tensor.matmul(
                            pu[:], wsu_t[k][:, j * P:(j + 1) * P], xbs[k][:],
                            start=(k == 0), stop=(k == KD - 1))
                    if skeleton:
                        a_ = as_pool.tile([P, T], BF16, tag="as")
                        nc.vector.tensor_copy(a_[0:1, 0:16], pg[0:1, 0:16])
                        nc.vector.tensor_copy(a_[0:1, 16:32], pu[0:1, 0:16])
                        as_tiles.append(xbs[j % KD])
                        continue
                    sg = sgup.tile([P, T], BF16, tag="sg")
                    nc.scalar.activation(sg[:], pg[:], Act.Sigmoid)
                    us = sgup.tile([P, T], BF16, tag="us")
                    nc.vector.tensor_copy(us[:], pu[:])
                    gu = sgup.tile([P, T], BF16, tag="gu")
                    nc.vector.tensor_tensor(gu[:], pg[:], us[:], Alu.mult)
                    a_ = as_pool.tile([P, T], BF16, tag="as")
                    nc.vector.tensor_tensor(a_[:], sg[:], gu[:], Alu.mult)
                    as_tiles.append(a_)

        # shared down-proj: k-outer accumulation into 6 pinned banks,
        # evicted to SBUF f32.
        shared_sb = []
        with tc.tile_pool(name="acc6", bufs=KD, space="PSUM") as acc_pool, \
             tc.tile_pool(name="wsd", bufs=10) as wsdp:
            acc = [acc_pool.tile([P, T], F32, tag="acc", name=f"acc{d}")
                   for d in range(KD)]
            wsd_cache = []
            for k in range(MS):
                if no_wdma and k >= 6:
                    t_ = wsd_cache[k % 6]
                else:
                    t_ = wsdp.tile([P, D], BF16, tag="wsd")
                    nc.sync.dma_start(t_[:], wsd[k * P:(k + 1) * P, :])
                    wsd_cache.append(t_)
                for d in range(KD):
                    nc.tensor.matmul(
                        acc[d][:], t_[:, d * P:(d + 1) * P], as_tiles[k][:],
                        start=(k == 0), stop=(k == MS - 1))
            for d in range(KD):
                s_ = shp.tile([P, T], F32, tag="sh")
                nc.vector.tensor_copy(s_[:], acc[d][:])
                shared_sb.append(s_)

        # =====================================================
        # Routed experts: per expert SwiGLU (gate folded in); per
        # (expert, d) down-proj groups churn through PSUM and are
        # accumulated into SBUF f32 tiles by the DVE.
        # =====================================================
        racp = ctx.enter_context(tc.tile_pool(name="racc", bufs=KD))
        racc = [racp.tile([P, T], F32, tag="racc", name=f"racc{d}")
                for d in range(KD)]
        with tc.tile_pool(name="wge", bufs=10) as wgep, \
             tc.tile_pool(name="wue", bufs=10) as wuep, \
             tc.tile_pool(name="wde", bufs=10) as wdep, \
             tc.tile_pool(name="psum_g", bufs=2, space="PSUM") as pgp, \
             tc.tile_pool(name="psum_u", bufs=2, space="PSUM") as pup, \
             tc.tile_pool(name="psum_d", bufs=3, space="PSUM") as pdp, \
             tc.tile_pool(name="gu", bufs=2) as gup, \
             tc.tile_pool(name="a2", bufs=MH + 2) as a2p:
            wgt0 = None
            for e in range(E):
                if no_wdma and e > 0:
                    wg_t, wu_t, wd_t = wgt0
                else:
                    wg_t, wu_t, wd_t = [], [], []
                    for k in range(KD):
                        t_ = wgep.tile([P, H], BF16, tag="wge")
                        nc.sync.dma_start(t_[:], wg[e, k * P:(k + 1) * P, :])
                        wg_t.append(t_)
                        t_ = wuep.tile([P, H], BF16, tag="wue")
                        nc.sync.dma_start(t_[:], wu[e, k * P:(k + 1) * P, :])
                        wu_t.append(t_)
                        t_ = wdep.tile([P, D], BF16, tag="wde")
                        nc.sync.dma_start(t_[:], wd[e, k * P:(k + 1) * P, :])
                        wd_t.append(t_)
                    wgt0 = (wg_t, wu_t, wd_t)
                a2_tiles = []
                for h in range(MH):
                    pg = pgp.tile([P, T], F32, tag="pg")
                    for k in range(KD):
                        nc.tensor.matmul(
                            pg[:], wg_t[k][:, h * P:(h + 1) * P], xbs[k][:],
                            start=(k == 0), stop=(k == KD - 1))
                    pu = pup.tile([P, T], F32, tag="pu")
                    for k in range(KD):
                        nc.tensor.matmul(
                            pu[:], wu_t[k][:, h * P:(h + 1) * P], xbs[k][:],
                            start=(k == 0), stop=(k == KD - 1))
                    if skeleton:
                        a2 = a2p.tile([P, T], BF16, tag="a2")
                        nc.vector.tensor_copy(a2[0:1, 0:16], pg[0:1, 0:16])
                        nc.vector.tensor_copy(a2[0:1, 16:32], pu[0:1, 0:16])
                        a2_tiles.append(xbs[h % KD])
                        continue
                    sg = gup.tile([P, T], BF16, tag="sg")
                    nc.scalar.activation(sg[:], pg[:], Act.Sigmoid)
                    us = gup.tile([P, T], BF16, tag="us")
                    nc.vector.tensor_tensor(us[:], pu[:], g_bcs[e][:],
                                            Alu.mult)
                    gu = gup.tile([P, T], BF16, tag="gu")
                    nc.vector.tensor_tensor(gu[:], pg[:], us[:], Alu.mult)
                    a2 = a2p.tile([P, T], BF16, tag="a2")
                    nc.vector.tensor_tensor(a2[:], sg[:], gu[:], Alu.mult)
                    a2_tiles.append(a2)
                for d in range(KD):
                    pd = pdp.tile([P, T], F32, tag="pd")
                    for k in range(MH):
                        nc.tensor.matmul(
                            pd[:], wd_t[k][:, d * P:(d + 1) * P],
                            a2_tiles[k][:],
                            start=(k == 0), stop=(k == MH - 1))
                    if e == 0:
                        nc.vector.tensor_copy(racc[d][:], pd[:])
                    else:
                        nc.vector.tensor_tensor(racc[d][:], pd[:],
                                                racc[d][:], Alu.add)

        # =====================================================
        # out = routed + alpha * (shared - routed)
        # =====================================================
        finp = ctx.enter_context(tc.tile_pool(name="fin", bufs=2))
        for d in range(KD):
            t1 = finp.tile([P, T], F32, tag="t1")
            nc.vector.tensor_tensor(t1[:], shared_sb[d][:], racc[d][:],
                                    Alu.subtract)
            t2 = finp.tile([P, T], F32, tag="t2")
            nc.vector.tensor_tensor(t2[:], t1[:], a_bc[:], Alu.mult)
            o_ = outp.tile([P, T], F32, tag="o")
            nc.vector.tensor_tensor(o_[:], t2[:], racc[d][:], Alu.add)
            nc.sync.dma_start(out_t[d * P:(d + 1) * P, :], o_[:])

    nc.compile()
    return nc


_NC_CACHE = None


def _get_program():
    global _NC_CACHE
    if _NC_CACHE is None:
        _NC_CACHE = _build_program()
    return _NC_CACHE


def make_in_maps(x, router_w, w_gate, w_up, w_down, ws_gate, ws_up, ws_down,
                 sg_w, sg_b):
    bf = mybir.dt.np(BF16)
    f32 = np.float32
    x2 = np.asarray(x, dtype=f32).reshape(B * S, D)
    shared = {
        "rw": np.asarray(router_w, dtype=f32),
        "sgw": np.asarray(sg_w, dtype=f32).reshape(D, 1),
        "sgb": np.asarray(sg_b, dtype=f32).reshape(1, 1),
        "wg": np.asarray(w_gate, dtype=f32).astype(bf),
        "wu": np.asarray(w_up, dtype=f32).astype(bf),
        "wd": np.asarray(w_down, dtype=f32).astype(bf),
        "wsg": np.asarray(ws_gate, dtype=f32).astype(bf),
        "wsu": np.asarray(ws_up, dtype=f32).astype(bf),
        "wsd": np.asarray(ws_down, dtype=f32).astype(bf),
    }
    in_maps = []
    for c in range(N_CORES):
        m = dict(shared)
        xtc = np.ascontiguousarray(x2[c * T:(c + 1) * T, :].T)
        m["xt"] = xtc
        m["xbi"] = xtc.astype(bf)
        in_maps.append(m)
    return in_maps


def assemble_out(results):
    cols = [np.asarray(results[c]["out_t"]) for c in range(N_CORES)]
    full_t = np.concatenate(cols, axis=1)  # [D, B*S]
    return np.ascontiguousarray(full_t.T).reshape(B, S, D).astype(np.float32)


def kernel(**inputs) -> np.ndarray:
    nc = _get_program()
    in_maps = make_in_maps(**inputs)
    res = run_bass_kernel_spmd(nc, in_maps, list(range(N_CORES)))
    return assemble_out(res.results)
